# revision 6
# baseline (speedup 1.0000x reference)
"""Trainium2 Bass kernel for the AttentionBlock problem.

Fixed problem shape: x [4, 64, 64, 64] fp32, GroupNorm(32 groups) ->
1x1 conv Q/K/V -> softmax(Q^T K / 8) -> V @ attn^T -> 1x1 conv + residual.

Sharding: 8 cores, core = 2*batch + query_half. Each core holds its batch's
full x (for K/V) and computes outputs for its 2048-query half.

Layout strategy (per core):
  - x, xn, K, Q live as [c=64 partitions, n free].
  - Scores are computed TRANSPOSED: S_T[k,q] = K_blk^T Q (contract c on
    partitions), softmax denominator comes free from a ones-row appended to
    V^T during the PV matmul (so no cross-partition reductions needed).
  - exp() runs on ScalarE directly out of PSUM, no max subtraction (scores
    are O(+-10) here; exp stays well inside fp32 range).
  - Output projection uses an augmented 65x65 Wo that carries the denominator
    row through, then 65x128 PE transposes put q on partitions so the
    1/denominator scaling is a per-partition tensor_scalar op.
"""

import sys
import types

import numpy as np
import ml_dtypes

import concourse.bass as bass
import concourse.mybir as mybir
import concourse.tile as tile
from concourse.vector_clock import ScopedClock

B, C, H, W = 4, 64, 64, 64
N = H * W            # 4096
NQ = N // 2          # queries per core
GROUPS = 32
EPS = 1e-5
KB = 32              # key blocks of 128
QC = 4               # query chunks of 512
F32 = mybir.dt.float32
BF16 = mybir.dt.bfloat16


# ---------------------------------------------------------------------------
# This container's walrus codegen rejects >1 sync wait on one instruction
# ("Too many sync wait commands") — split extra waits onto preceding same-
# engine NOPs (engines execute in order, so semantics are preserved), and do
# the same for the TileContext tail drain.
def _install_drain_patch():
    if getattr(tile.TileContext, "_drain_patch_installed", False):
        return

    orig_commit = tile.TileContext._commit_instruction

    def _split_commit(self, inst, lazy_reg_writes=True):
        si = getattr(inst, "sync_info", None)
        if (
            si is not None
            and len(si.on_wait) > 1
            and inst.engine != mybir.EngineType.Unassigned
        ):
            waits = list(si.on_wait)
            inst.sync_info = mybir.SyncInfo(
                on_wait=waits[-1:], on_update=list(si.on_update)
            )
            for w in waits[:-1]:
                nop = mybir.InstNoOp(
                    name=self.nc.get_next_instruction_name(),
                    sync_info=mybir.SyncInfo(on_wait=[w], on_update=[]),
                    bass_nofuse=True,
                    engine=inst.engine,
                )
                orig_commit(self, nop, lazy_reg_writes=False)
        orig_commit(self, inst, lazy_reg_writes)

    def _patched(self, tick_clock, wait_clock):
        nc = self.nc
        drain_inst = nc.sync.drain()
        wait_clock.add_sem_waits(
            drain_inst.ins, ScopedClock({None: tick_clock.global_clock})
        )
        si = drain_inst.ins.sync_info
        if si is not None and len(si.on_wait) > 1:
            waits = list(si.on_wait)
            drain_inst.ins.sync_info = mybir.SyncInfo(
                on_wait=waits[:1], on_update=list(si.on_update)
            )
            for i in range(1, len(waits)):
                extra = nc.sync.drain()
                extra.ins.sync_info = mybir.SyncInfo(
                    on_wait=waits[i : i + 1], on_update=[]
                )
        nc.all_engine_barrier()
        assert self.sems is not None
        popped = nc._tile_sem_poison_stack.pop()
        assert popped is self._sem_poison
        nc.clear_and_free_semaphores(list(self.sems.allocated().values()))
        nc.all_engine_barrier()

    tile.TileContext._commit_instruction = _split_commit
    tile.TileContext._drain_and_barrier = _patched
    tile.TileContext._drain_patch_installed = True


def build_nc():
    _install_drain_patch()
    nc = bass.Bass()

    # per-core data
    x_d = nc.dram_tensor("x", [C, N], F32, kind="ExternalInput")
    xq_d = nc.dram_tensor("xq", [C, NQ], F32, kind="ExternalInput")
    xt_d = nc.dram_tensor("xt", [NQ, C], F32, kind="ExternalInput")
    # replicated weights / constants
    wq_d = nc.dram_tensor("wq_t", [C, C], BF16, kind="ExternalInput")
    wk_d = nc.dram_tensor("wk_t", [C, C], BF16, kind="ExternalInput")
    wv_d = nc.dram_tensor("wv_t", [C, C], BF16, kind="ExternalInput")
    waug_d = nc.dram_tensor("w_aug", [C + 1, C + 1], BF16, kind="ExternalInput")
    ident_d = nc.dram_tensor("ident65", [C + 1, C + 1], F32, kind="ExternalInput")
    pair_d = nc.dram_tensor("pairmat", [C, C], F32, kind="ExternalInput")
    gb_d = nc.dram_tensor("gb", [C, 4], F32, kind="ExternalInput")  # gamma,beta,bq,bk
    bo_d = nc.dram_tensor("bo_bc", [128, C], F32, kind="ExternalInput")
    y_d = nc.dram_tensor("y", [NQ, C], F32, kind="ExternalOutput")

    with tile.TileContext(nc) as tc:
        with (
            tc.tile_pool(name="const", bufs=1) as const,
            tc.tile_pool(name="big", bufs=1) as big,
            tc.tile_pool(name="stats", bufs=2) as stats,
            tc.tile_pool(name="pt", bufs=4) as ptp,
            tc.tile_pool(name="tail", bufs=2) as tailp,
            tc.tile_pool(name="yp", bufs=3) as yp,
            tc.tile_pool(name="xtp", bufs=3) as xtp,
            tc.tile_pool(name="sps", bufs=2, space="PSUM") as sps,
            tc.tile_pool(name="ops", bufs=4, space="PSUM") as ops,
            tc.tile_pool(name="ztp", bufs=2, space="PSUM") as ztp,
        ):
            # ---- load constants
            wq = const.tile([C, C], BF16, tag="wq")
            wk = const.tile([C, C], BF16, tag="wk")
            wv = const.tile([C, C], BF16, tag="wv")
            waug = const.tile([C + 1, C + 1], BF16, tag="waug")
            ident = const.tile([C + 1, C + 1], F32, tag="ident")
            pair = const.tile([C, C], F32, tag="pair")
            gb = const.tile([C, 4], F32, tag="gb")
            bo_bc = const.tile([128, C], F32, tag="bo")
            nc.sync.dma_start(out=wq, in_=wq_d[:, :])
            nc.sync.dma_start(out=wk, in_=wk_d[:, :])
            nc.sync.dma_start(out=wv, in_=wv_d[:, :])
            nc.sync.dma_start(out=waug, in_=waug_d[:, :])
            nc.sync.dma_start(out=ident, in_=ident_d[:, :])
            nc.sync.dma_start(out=pair, in_=pair_d[:, :])
            nc.sync.dma_start(out=gb, in_=gb_d[:, :])
            nc.sync.dma_start(out=bo_bc, in_=bo_d[:, :])
            gamma = gb[:, 0:1]
            beta = gb[:, 1:2]
            bq_col = gb[:, 2:3]
            bk_col = gb[:, 3:4]

            # ---- load x; bn_stats per 512-chunk as chunks arrive
            x_sb = big.tile([C, N], F32, tag="x")
            xq_sb = big.tile([C, NQ], F32, tag="xq")
            st = stats.tile([C, 8, 6], F32, tag="bnst")
            for j in range(8):
                sl = bass.ts(j, 512)
                nc.sync.dma_start(out=x_sb[:, sl], in_=x_d[:, sl])
                nc.vector.bn_stats(out=st[:, j, :], in_=x_sb[:, sl])
            for j in range(4):
                sl = bass.ts(j, 512)
                nc.sync.dma_start(out=xq_sb[:, sl], in_=xq_d[:, sl])
            mv = stats.tile([C, 2], F32, tag="mv")
            nc.vector.bn_aggr(out=mv, in_=st)
            # me2 = [mean, var + mean^2] per channel
            me2 = stats.tile([C, 2], F32, tag="me2")
            nc.vector.tensor_copy(out=me2[:, 0:1], in_=mv[:, 0:1])
            m2 = stats.tile([C, 1], F32, tag="m2")
            nc.vector.tensor_mul(out=m2, in0=mv[:, 0:1], in1=mv[:, 0:1])
            nc.vector.tensor_add(out=me2[:, 1:2], in0=mv[:, 1:2], in1=m2)
            # group (channel-pair) means of [mean, E[x^2]] via tiny matmul
            gps = ztp.tile([C, 2], F32, tag="zt")
            nc.tensor.matmul(out=gps, lhsT=pair, rhs=me2, start=True, stop=True)
            mean_g = stats.tile([C, 1], F32, tag="meang")
            nc.vector.tensor_copy(out=mean_g, in_=gps[:, 0:1])
            varg = stats.tile([C, 1], F32, tag="varg")
            nc.vector.tensor_mul(out=varg, in0=mean_g, in1=mean_g)
            nc.vector.tensor_tensor(
                out=varg, in0=gps[:, 1:2], in1=varg, op=mybir.AluOpType.subtract
            )
            # rstd = 1/sqrt(var+eps);  s = rstd*gamma;  t = beta - mean*s
            eps_t = stats.tile([C, 1], F32, tag="eps")
            nc.vector.memset(eps_t, EPS)
            nc.scalar.activation(
                out=varg, in_=varg, func=mybir.ActivationFunctionType.Sqrt, bias=eps_t
            )
            rstd = stats.tile([C, 1], F32, tag="rstd")
            nc.vector.reciprocal(out=rstd, in_=varg)
            s_col = stats.tile([C, 1], F32, tag="scol")
            nc.vector.tensor_mul(out=s_col, in0=rstd, in1=gamma)
            t_col = stats.tile([C, 1], F32, tag="tcol")
            nc.vector.tensor_mul(out=t_col, in0=mean_g, in1=s_col)
            nc.vector.tensor_tensor(
                out=t_col, in0=beta, in1=t_col, op=mybir.AluOpType.subtract
            )

            # ---- normalized activations (bf16)
            xn = big.tile([C, N], BF16, tag="xn")
            xnq = big.tile([C, NQ], BF16, tag="xnq")
            for j in range(8):
                sl = bass.ts(j, 512)
                nc.vector.tensor_scalar(
                    out=xn[:, sl], in0=x_sb[:, sl], scalar1=s_col, scalar2=t_col,
                    op0=mybir.AluOpType.mult, op1=mybir.AluOpType.add,
                )
            for j in range(4):
                sl = bass.ts(j, 512)
                nc.vector.tensor_scalar(
                    out=xnq[:, sl], in0=xq_sb[:, sl], scalar1=s_col, scalar2=t_col,
                    op0=mybir.AluOpType.mult, op1=mybir.AluOpType.add,
                )

            # ---- K = Wk@xn + bk (full), Q = Wq@xnq + bq (this core's half)
            k_sb = big.tile([C, N], BF16, tag="k")
            q_sb = big.tile([C, NQ], BF16, tag="q")
            for j in range(8):
                sl = bass.ts(j, 512)
                ps = sps.tile([C, 512], F32, tag="sps")
                nc.tensor.matmul(out=ps, lhsT=wk, rhs=xn[:, sl], start=True, stop=True)
                nc.scalar.activation(
                    out=k_sb[:, sl], in_=ps,
                    func=mybir.ActivationFunctionType.Identity, bias=bk_col,
                )
            for j in range(4):
                sl = bass.ts(j, 512)
                ps = sps.tile([C, 512], F32, tag="sps")
                nc.tensor.matmul(out=ps, lhsT=wq, rhs=xnq[:, sl], start=True, stop=True)
                nc.scalar.activation(
                    out=q_sb[:, sl], in_=ps,
                    func=mybir.ActivationFunctionType.Identity, bias=bq_col,
                )
            # ---- V^T blocks [128, 65] with ones column (denominator trick)
            vt = big.tile([128, KB, C + 1], BF16, tag="vt")
            for kb in range(KB):
                ps = ztp.tile([128, C], F32, tag="zt")
                nc.tensor.matmul(
                    out=ps, lhsT=xn[:, bass.ts(kb, 128)], rhs=wv, start=True, stop=True
                )
                nc.vector.tensor_copy(out=vt[:, kb, 0:C], in_=ps)
                nc.vector.memset(vt[:, kb, C : C + 1], 1.0)

            # ---- main attention loop
            o_tiles = [
                ops.tile([C + 1, 512], F32, tag="o", name=f"o{qc}")
                for qc in range(QC)
            ]
            for kb in range(KB):
                kblk = k_sb[:, bass.ts(kb, 128)]
                for qc in range(QC):
                    s_ps = sps.tile([128, 512], F32, tag="sps")
                    nc.tensor.matmul(
                        out=s_ps, lhsT=kblk, rhs=q_sb[:, bass.ts(qc, 512)],
                        start=True, stop=True,
                    )
                    p_sb = ptp.tile([128, 512], BF16, tag="p")
                    nc.scalar.activation(
                        out=p_sb, in_=s_ps,
                        func=mybir.ActivationFunctionType.Exp, scale=0.125,
                    )
                    nc.tensor.matmul(
                        out=o_tiles[qc], lhsT=vt[:, kb, :], rhs=p_sb,
                        start=(kb == 0), stop=(kb == KB - 1),
                        skip_group_check=True,
                    )

            # ---- tail: project through augmented Wo, transpose, normalize,
            #      add residual, store
            for qc in range(QC):
                ou = tailp.tile([C + 1, 512], BF16, tag="ou")
                nc.vector.tensor_copy(out=ou, in_=o_tiles[qc])
                z_ps = sps.tile([C + 1, 512], F32, tag="sps")
                nc.tensor.matmul(out=z_ps, lhsT=waug, rhs=ou, start=True, stop=True)
                z_sb = tailp.tile([C + 1, 512], F32, tag="z")
                nc.vector.tensor_copy(out=z_sb, in_=z_ps)
                for jb in range(4):
                    zt_ps = ztp.tile([128, C + 1], F32, tag="zt")
                    nc.tensor.transpose(
                        out=zt_ps, in_=z_sb[:, bass.ts(jb, 128)], identity=ident
                    )
                    qrow = qc * 512 + jb * 128
                    xt_t = xtp.tile([128, C], F32, tag="xt")
                    nc.sync.dma_start(out=xt_t, in_=xt_d[qrow : qrow + 128, :])
                    r = yp.tile([128, 1], F32, tag="r")
                    nc.vector.reciprocal(out=r, in_=zt_ps[:, C : C + 1])
                    y1 = yp.tile([128, C], F32, tag="y1")
                    nc.vector.scalar_tensor_tensor(
                        out=y1, in0=zt_ps[:, 0:C], scalar=r, in1=xt_t,
                        op0=mybir.AluOpType.mult, op1=mybir.AluOpType.add,
                    )
                    y2 = yp.tile([128, C], F32, tag="y2")
                    nc.vector.tensor_add(out=y2, in0=y1, in1=bo_bc)
                    nc.sync.dma_start(out=y_d[qrow : qrow + 128, :], in_=y2)
    return nc


_NC = None


def _get_nc():
    global _NC
    if _NC is None:
        _NC = build_nc()
    return _NC


def _prep_maps(x, Wq, bq, Wk, bk, Wv, bv, Wo, bo, gamma, beta):
    bf = ml_dtypes.bfloat16
    wq_t = np.ascontiguousarray(Wq.T).astype(bf)
    wk_t = np.ascontiguousarray(Wk.T).astype(bf)
    wv_t = np.ascontiguousarray(Wv.T).astype(bf)
    w_aug = np.zeros((C + 1, C + 1), np.float32)
    w_aug[:C, :C] = Wo.T
    w_aug[C, :C] = Wo @ bv
    w_aug[C, C] = 1.0
    w_aug = w_aug.astype(bf)
    ident65 = np.eye(C + 1, dtype=np.float32)
    pairmat = np.zeros((C, C), np.float32)
    for k in range(C):
        for m in range(C):
            if k // 2 == m // 2:
                pairmat[k, m] = 0.5
    gb = np.stack([gamma, beta, bq, bk], axis=1).astype(np.float32)
    bo_bc = np.tile(bo[None, :], (128, 1)).astype(np.float32)

    shared = dict(
        wq_t=wq_t, wk_t=wk_t, wv_t=wv_t, w_aug=w_aug, ident65=ident65,
        pairmat=pairmat, gb=gb, bo_bc=bo_bc,
    )
    in_maps = []
    for core in range(8):
        b, half = core // 2, core % 2
        xm = np.ascontiguousarray(x[b].reshape(C, N)).astype(np.float32)
        xqm = np.ascontiguousarray(xm[:, half * NQ : (half + 1) * NQ])
        xtm = np.ascontiguousarray(xm.T[half * NQ : (half + 1) * NQ, :])
        in_maps.append(dict(shared, x=xm, xq=xqm, xt=xtm))
    return in_maps


def run(inputs, trace=False):
    from concourse.bass_utils import run_bass_kernel_spmd

    inputs = {k: np.asarray(v) for k, v in inputs.items()}
    nc = _get_nc()
    in_maps = _prep_maps(**inputs)
    res = run_bass_kernel_spmd(
        nc, in_maps, core_ids=list(range(8)), trace=trace
    )
    out = np.empty((B, C, N), np.float32)
    for core in range(8):
        b, half = core // 2, core % 2
        out[b][:, half * NQ : (half + 1) * NQ] = res.results[core]["y"].T
    return out.reshape(B, C, H, W), res


def kernel(**inputs):
    out, _ = run(inputs, trace=False)
    return out
